# revision 25
# baseline (speedup 1.0000x reference)
"""Multi-head attention (B=4, S=2048, D=512, H=8) on 8 trn2 NeuronCores.

Sharding: core c = (batch b = c//2, query-half qh = c%2). Each core computes
the full attention output for 1024 query rows of one batch element.

Single fully-pipelined phase (measured ~131us vs ~151us for the phased
version). The attention loop is ACT-bound: exp of 72 [128,1024] logit tiles
at ~1.1us each is the critical resource; everything else hides under it.
The 8 (query-chunk, head-pair) windows run as one continuous software
pipeline (AV matmul pairs lag their exp by 2 steps, carrying the pipeline
across window boundaries with no ACT gap). Projection and O-projection
matmul "units" are injected into per-step slots of the PE stream via a
deadline-scheduled (EDF, latest-fit) plan, filling the PE's exp-wait slack.
PSUM (8 banks): logits ring 2x[128,1024] + two alternating av tag-sets
(A,B)x2 -- window w+1 uses the opposite set from w so w's normalization
chain (copy denom row -> reciprocal -> gpsimd partition broadcast ->
multiply) never blocks w+1's AV matmuls; injected units borrow the set the
current window is not using (slots >= 4). The last window pre-accumulates
all four O(1) blocks over head-pairs 0..2 so only one matmul + drain per
block remains after the final normalization. DMA pieces are issued in
consumption order (weights, first activation halves, then the rest);
warm-up matmuls keep the PE HAM un-throttled from ~6us. Clock caveat: an
earlier variant that packed the schedule differently triggered a chip
power-state downclock (PE 2.4->2.0GHz, ACT 1.2->1.0GHz); this layout
measures at full clocks.

Device-side scheme (unchanged):
  - host supplies transposed activations (X^T layouts); Q^T/K^T projections
    with per-partition bias in the drain; V in natural layout [128, H, DH+1]
    with a ones column (softmax denominator rides in the AV matmul, M=65).
  - logits transposed: lg^T[s_k, q], two heads per PE pass (row-packed K=64).
  - exp on ACT: w = exp(0.125*lg + mb); masked/padded keys get bias -1e9.
  - AV accumulated per head over key blocks; normalization via reciprocal +
    partition broadcast; O projection streamed to HBM per 128 rows.

Masked keys (mask==1) are compacted away on the host (exact), keys padded to
a multiple of 128 with -1e9 mask bias.
"""

import os
import numpy as np

B, S, D, H = 4, 2048, 512, 8
DH = D // H
NCORE = 8
SQ = S // 2  # queries per core
SCALE = 1.0 / float(np.sqrt(DH))

_BUILT = {}


def _chunks(total, step):
    out = []
    c0 = 0
    while c0 < total:
        out.append((c0, min(step, total - c0)))
        c0 += step
    return out


def build_bass(s_pad, mm_dtype="bf16"):
    import concourse.bass as bass  # noqa: F401
    import concourse.mybir as mybir
    import concourse.tile as tile
    from concourse import bacc
    from contextlib import ExitStack

    f32 = mybir.dt.float32
    mmdt = {
        "bf16": mybir.dt.bfloat16,
        "f32r": mybir.dt.float32r,
        "f32": mybir.dt.float32,
    }[mm_dtype]
    EXP = mybir.ActivationFunctionType.Exp

    nsk = s_pad // 128

    nc = bacc.Bacc(
        "TRN2",
        target_bir_lowering=False,
        debug=False,
        enable_asserts=False,
        num_devices=NCORE,
    )

    KW, QW = 4 * s_pad, 4 * SQ
    d_bk_blob = nc.dram_tensor("blob_k", [128, 2048 + KW], mmdt, kind="ExternalInput").ap()
    d_bq_blob = nc.dram_tensor("blob_q", [128, 2048 + QW], mmdt, kind="ExternalInput").ap()
    d_bv_blob = nc.dram_tensor("blob_v", [128, 2048 + KW + D + 8], mmdt, kind="ExternalInput").ap()
    d_bo_blob = nc.dram_tensor("blob_o", [128, 2048 + D], mmdt, kind="ExternalInput").ap()
    d_mb = nc.dram_tensor("mb", [128, nsk], f32, kind="ExternalInput").ap()
    d_bq = nc.dram_tensor("bq_pp", [128, 4], f32, kind="ExternalInput").ap()
    d_bk = nc.dram_tensor("bk_pp", [128, 4], f32, kind="ExternalInput").ap()
    d_out = nc.dram_tensor("out", [SQ, D], f32, kind="ExternalOutput").ap()

    with tile.TileContext(nc) as tc, ExitStack() as ctx, nc.allow_low_precision(
        "matmul operands held in bf16 (tolerance 2e-2; measured ~6e-3)"
    ):
        sb = ctx.enter_context(tc.tile_pool(name="sb", bufs=1))
        # PSUM budget (16KB/partition): lg 2x[128,1024]f32 = 8KB;
        # av sets 0/1 x (A,B) 1 buf x [128,512]f32 = 8KB. Injected units
        # borrow slots from the av set the current window is NOT using.
        ps_lg = ctx.enter_context(tc.tile_pool(name="pslg", bufs=2, space="PSUM"))
        ps_av = ctx.enter_context(tc.tile_pool(name="psav", bufs=1, space="PSUM"))
        wexp_p = ctx.enter_context(tc.tile_pool(name="wexp", bufs=4))
        osb_p = ctx.enter_context(tc.tile_pool(name="osb", bufs=4))
        r_p = ctx.enter_context(tc.tile_pool(name="rp", bufs=2))

        # ---- persistent SBUF tiles ----
        blk = sb.tile([128, 2048 + KW], mmdt, tag="blk", name="blk")
        blq = sb.tile([128, 2048 + QW], mmdt, tag="blq", name="blq")
        blv = sb.tile([128, 2048 + KW + D + 8], mmdt, tag="blv", name="blv")
        blo = sb.tile([128, 2048 + D], mmdt, tag="blo", name="blo")
        bk = sb.tile([128, 4], f32, tag="bk", name="bk")
        bq = sb.tile([128, 4], f32, tag="bq", name="bq")
        mb = sb.tile([128, nsk], f32, tag="mb", name="mb")
        kT = [sb.tile([128, s_pad], mmdt, tag=f"kT{j}", name=f"kT{j}") for j in range(4)]
        qT = [sb.tile([128, SQ], mmdt, tag=f"qT{j}", name=f"qT{j}") for j in range(4)]
        attnN = [
            sb.tile([128, SQ], mmdt, tag=f"attnN{pr}", name=f"attnN{pr}")
            for pr in range(4)
        ]
        v = [
            sb.tile([128, H, DH + 1], mmdt, tag=f"v{t}", name=f"v{t}")
            for t in range(nsk)
        ]
        wu = sb.tile([128, 512], mmdt, tag="wu", name="wu")

        bvb = blv[:, 2048 + KW : 2048 + KW + D]
        ones8 = blv[:, 2048 + KW + D : 2048 + KW + D + 8]
        bob = blo[:, 2048 : 2048 + D]

        # ---- DMA emission, consumption order ----
        X = 2048  # x-region offset inside k/q/v blobs
        h_k = min(1024, s_pad)
        p_v0 = min(5 * 128, s_pad)  # xv piece 0 covers t=0..4

        def ld(dst, src):
            nc.sync.dma_start(dst, src)

        ld(bk[:], d_bk[:])
        ld(bq[:], d_bq[:])
        ld(mb[:], d_mb[:])
        for c0, cw in _chunks(2048, 1024):  # wk
            ld(blk[:, c0 : c0 + cw], d_bk_blob[:, c0 : c0 + cw])
        for c0, cw in _chunks(2048, 1024):  # wq
            ld(blq[:, c0 : c0 + cw], d_bq_blob[:, c0 : c0 + cw])
        for dk in range(4):  # xk first halves
            o = X + dk * s_pad
            ld(blk[:, o : o + h_k], d_bk_blob[:, o : o + h_k])
        for dk in range(4):  # xq first halves
            o = X + dk * SQ
            ld(blq[:, o : o + 512], d_bq_blob[:, o : o + 512])
        for c0, cw in _chunks(2048, 1024):  # wv
            ld(blv[:, c0 : c0 + cw], d_bv_blob[:, c0 : c0 + cw])
        ld(blv[:, X + KW :], d_bv_blob[:, X + KW :])  # v bias + ones
        for dk in range(4):  # xv piece 0 (t=0..4)
            o = X + dk * s_pad
            ld(blv[:, o : o + p_v0], d_bv_blob[:, o : o + p_v0])
        if h_k < s_pad:  # xk second halves
            for dk in range(4):
                o = X + dk * s_pad + h_k
                e = X + dk * s_pad + s_pad
                ld(blk[:, o:e], d_bk_blob[:, o:e])
        if p_v0 < s_pad:  # xv piece 1
            for dk in range(4):
                o = X + dk * s_pad + p_v0
                e = X + dk * s_pad + s_pad
                ld(blv[:, o:e], d_bv_blob[:, o:e])
        for dk in range(4):  # xq second halves
            o = X + dk * SQ + 512
            ld(blq[:, o : o + 512], d_bq_blob[:, o : o + 512])
        for c0, cw in _chunks(2048 + D, 1024):  # wo + output bias (~55us in)
            ld(blo[:, c0 : c0 + cw], d_bo_blob[:, c0 : c0 + cw])

        # ---- borrowed-psum allocator for injected units ----
        bctx = {"set": 1, "tog": 0, "n": 0}

        def unit_ps(shape=None):
            tag = f"av{bctx['set']}{'AB'[bctx['tog']]}"
            bctx["tog"] ^= 1
            bctx["n"] += 1
            return ps_av.tile(
                shape or [128, 512], f32, tag=tag, name=f"ups{bctx['n']}"
            )

        # ---- PE warm-up (no data deps; runs during preamble/DMA) ----
        nc.vector.memset(wu[:], 0.0)
        for i in range(14):
            wps = unit_ps()
            nc.tensor.matmul(wps[:], lhsT=wu[:, 0:128], rhs=wu[:], start=True, stop=True)

        # ---- projection emitters ----
        def kq_chunk(which, j, c0, cw):
            blob, bias_t, dst = (blk, bk, kT) if which == "k" else (blq, bq, qT)
            sp = s_pad if which == "k" else SQ
            ps = unit_ps()
            for dk in range(4):
                nc.tensor.matmul(
                    ps[:, 0:cw],
                    lhsT=blob[:, dk * 512 + j * 128 : dk * 512 + (j + 1) * 128],
                    rhs=blob[:, X + dk * sp + c0 : X + dk * sp + c0 + cw],
                    start=(dk == 0),
                    stop=(dk == 3),
                )
            nc.vector.tensor_scalar_add(
                dst[j][:, c0 : c0 + cw], ps[:, 0:cw], bias_t[:, j : j + 1]
            )

        def v_proj(t):
            ps = unit_ps()
            for dk in range(4):
                nc.tensor.matmul(
                    ps[:],
                    lhsT=blv[:, X + dk * s_pad + t * 128 : X + dk * s_pad + (t + 1) * 128],
                    rhs=blv[:, dk * 512 : (dk + 1) * 512],
                    start=(dk == 0),
                    stop=(dk == 3),
                )
            nc.vector.tensor_copy(
                v[t][:, :, DH : DH + 1], ones8.rearrange("p (h o) -> p h o", o=1)
            )
            nc.vector.scalar_tensor_tensor(
                v[t][:, :, 0:DH],
                ps[:].rearrange("p (h d) -> p h d", h=H),
                1.0,
                bvb.rearrange("p (h d) -> p h d", h=H),
                op0=mybir.AluOpType.mult,
                op1=mybir.AluOpType.add,
            )

        def o_drain(qc, qt, ops):
            qq = qc * 512 + qt * 128
            osb = osb_p.tile([128, D], f32, tag="osb", name=f"osb{qc}_{qt}")
            nc.vector.scalar_tensor_tensor(
                osb[:],
                ops[:],
                1.0,
                bob,
                op0=mybir.AluOpType.mult,
                op1=mybir.AluOpType.add,
            )
            nc.sync.dma_start(d_out[qq : qq + 128, :], osb[:])

        def o_unit(qc, qt, ps=None, prs=range(4), stop=True):
            qq = qc * 512 + qt * 128
            if ps is None:
                ps = unit_ps()
            for pr2 in prs:
                nc.tensor.matmul(
                    ps[:],
                    lhsT=attnN[pr2][:, qq : qq + 128],
                    rhs=blo[:, pr2 * 512 : (pr2 + 1) * 512],
                    start=(pr2 == 0),
                    stop=(pr2 == 3 and stop),
                )
            if stop:
                o_drain(qc, qt, ps)
            return ps

        # ---- unit schedule over the flat step sequence ----
        # Steps: 8 windows x nsk key blocks, one exp tile per step. AV pairs
        # lag two steps behind their exp (decouples av from the lead-in DMA
        # and carries the pipeline across window boundaries). Injected units
        # get (earliest, deadline) step constraints and are placed EDF.
        kch = _chunks(s_pad, 512)
        windows = [(qc, pr) for qc in range(2) for pr in range(4)]
        NW = len(windows)
        nstep = NW * nsk

        units = []  # (unit, earliest_step, deadline_step)
        E_XKH1 = 3 if h_k < s_pad else 0  # xk 2nd half lands ~step 3
        E_XVP1 = 4 if p_v0 < s_pad else 0
        E_XQH1 = 8  # xq 2nd half lands ~step 8
        for t in range(nsk):
            e = 0 if t < 5 else E_XVP1
            units.append((("v", t), e, max(t, e)))  # used by av at step t+2
        for j in range(4):
            for ci, (c0, cw) in enumerate(kch):
                if j == 0 and ci == 0:
                    continue  # up-front
                use = 0 * nsk + c0 // 128 if j == 0 else j * nsk + c0 // 128
                e = E_XKH1 if c0 >= h_k else 0
                units.append((("k", j, c0, cw), e, max(use - 2, e)))
            if j > 0:
                units.append((("q", j, 0, 512), 0, j * nsk - 2))
            units.append((("q", j, 512, 512), E_XQH1, max((4 + j) * nsk - 2, E_XQH1)))

        pinned = {}  # step -> list of units
        for qt in range(4):  # O(0) inside window (1,0), after finish(0,3)
            pinned.setdefault(4 * nsk + min(5 + qt, nsk - 1), []).append(("o", 0, qt))
        for qt in (0, 1):  # O(1) qt0/qt1 partial pre-accumulation in (1,3)
            pinned.setdefault(7 * nsk + min(5 + qt, nsk - 1), []).append(("o1p", 1, qt))

        # usable slots: window 0 all steps; later windows steps >= 4 (the
        # borrowed av set is released by the previous window's normalization
        # chain only ~3 steps in)
        free = []
        for wi in range(NW):
            s0 = 0 if wi == 0 else 4
            for t in range(s0, nsk):
                st = wi * nsk + t
                if st not in pinned:
                    free.append(st)
        cap = {st: (2 if st >= 3 else 1) if st < nsk else 1 for st in free}
        load = {st: 0 for st in free}
        plan = {st: [] for st in range(nstep)}
        for st, us in pinned.items():
            plan[st] = list(us)
        for u, e, dl in sorted(units, key=lambda x: x[2]):
            cands = [s for s in free if e <= s <= dl and load[s] < cap[s]]
            if cands:
                s = max(cands)  # latest-fit: don't front-load early windows
            else:
                cands = [s for s in free if e <= s <= dl]
                s = max(cands) if cands else min(s for s in free if s >= e)
            plan[s].append(u)
            load[s] += 1

        o1_ps = {}

        def run_unit(u):
            if u[0] == "v":
                v_proj(u[1])
            elif u[0] in ("k", "q"):
                kq_chunk(u[0], u[1], u[2], u[3])
            elif u[0] == "o":
                o_unit(u[1], u[2])
            elif u[0] == "o1p":  # O(1) partial pre-accumulation (pr 0..2)
                o1_ps[u[2]] = o_unit(u[1], u[2], prs=range(3), stop=False)

        # ---- up-front projections ----
        kq_chunk("k", 0, 0, 512)
        kq_chunk("q", 0, 0, 512)

        # ---- continuous attention pipeline ----
        AVLAG = 2
        wctx = {}  # wi -> (avA, avB, hA, hB, q0)

        def emit_av(wi, t, wx):
            avA, avB, hA, hB, q0 = wctx[wi]
            last = t == nsk - 1
            nc.tensor.matmul(
                avA[0:65, :],
                lhsT=v[t][:, hA : hA + 1, 0 : DH + 1],
                rhs=wx[:, 0:512],
                start=(t == 0),
                stop=last,
            )
            nc.tensor.matmul(
                avB[0:65, :],
                lhsT=v[t][:, hB : hB + 1, 0 : DH + 1],
                rhs=wx[:, 512:1024],
                start=(t == 0),
                stop=last,
            )
            if last:
                finish(wi)

        def finish(wi):
            avA, avB, hA, hB, q0 = wctx.pop(wi)
            qc, pr = windows[wi]
            # normalization (reciprocal needs SBUF input: copy denom rows out)
            dsA = r_p.tile([1, 512], f32, tag="dsA", name=f"dsA{qc}{pr}")
            dsB = r_p.tile([1, 512], f32, tag="dsB", name=f"dsB{qc}{pr}")
            nc.vector.tensor_copy(dsA[0:1, :], avA[64:65, :])
            nc.vector.tensor_copy(dsB[0:1, :], avB[64:65, :])
            rfA = r_p.tile([1, 512], f32, tag="rfA", name=f"rfA{qc}{pr}")
            rfB = r_p.tile([1, 512], f32, tag="rfB", name=f"rfB{qc}{pr}")
            nc.vector.reciprocal_approx_fast(rfA[0:1, :], dsA[0:1, :])
            nc.vector.reciprocal_approx_fast(rfB[0:1, :], dsB[0:1, :])
            bcsA = r_p.tile([64, 512], f32, tag="bcsA", name=f"bcsA{qc}{pr}")
            bcsB = r_p.tile([64, 512], f32, tag="bcsB", name=f"bcsB{qc}{pr}")
            nc.gpsimd.partition_broadcast(bcsA[0:64, :], rfA[0:1, :], channels=64)
            nc.gpsimd.partition_broadcast(bcsB[0:64, :], rfB[0:1, :], channels=64)
            nc.vector.tensor_mul(
                attnN[pr][0:64, q0 : q0 + 512], avA[0:64, :], bcsA[0:64, :]
            )
            nc.vector.tensor_mul(
                attnN[pr][64:128, q0 : q0 + 512], avB[0:64, :], bcsB[0:64, :]
            )

        hist = []  # emitted (wi, t, wx) awaiting their av pair
        for step in range(nstep):
            wi, t = divmod(step, nsk)
            qc, pr = windows[wi]
            if t == 0:
                st = wi % 2
                wctx[wi] = (
                    ps_av.tile([65, 512], f32, tag=f"av{st}A", name=f"avA{qc}{pr}"),
                    ps_av.tile([65, 512], f32, tag=f"av{st}B", name=f"avB{qc}{pr}"),
                    2 * pr,
                    2 * pr + 1,
                    qc * 512,
                )
            bctx["set"] = 1 - (wi % 2)
            lg = ps_lg.tile([128, 1024], f32, tag="lg", name=f"lg{qc}{pr}_{t}")
            q0 = qc * 512
            nc.tensor.matmul(
                lg[:, 0:512],
                lhsT=kT[pr][0:64, t * 128 : (t + 1) * 128],
                rhs=qT[pr][0:64, q0 : q0 + 512],
                start=True,
                stop=True,
            )
            nc.tensor.matmul(
                lg[:, 512:1024],
                lhsT=kT[pr][64:128, t * 128 : (t + 1) * 128],
                rhs=qT[pr][64:128, q0 : q0 + 512],
                start=True,
                stop=True,
            )
            wx = wexp_p.tile([128, 1024], mmdt, tag="wexp", name=f"wx{qc}{pr}_{t}")
            nc.scalar.activation(wx[:], lg[:], EXP, bias=mb[:, t : t + 1], scale=SCALE)
            hist.append((wi, t, wx))
            lag = 1 if step >= nstep - 2 else AVLAG
            while len(hist) > lag:
                emit_av(*hist.pop(0))
            for u in plan[step]:
                run_unit(u)
        # qt2 partial on the first now-free lg slot (runs while the last
        # exps drain), then flush the remaining av pairs + final finish
        o1_ps[2] = o_unit(
            1, 2,
            ps=ps_lg.tile([128, 1024], f32, tag="lg", name="o1f2")[:, 0:512],
            prs=range(3), stop=False,
        )
        for h in hist:
            emit_av(*h)
        o1_ps[3] = o_unit(
            1, 3,
            ps=ps_lg.tile([128, 1024], f32, tag="lg", name="o1f3")[:, 0:512],
            prs=range(3), stop=False,
        )

        # ---- tail: finalize O(1): one matmul (pr=3) + drain per qt ----
        for qt in range(4):
            ps = o1_ps[qt]
            qq = 512 + qt * 128
            nc.tensor.matmul(
                ps[:],
                lhsT=attnN[3][:, qq : qq + 128],
                rhs=blo[:, 3 * 512 : 4 * 512],
                start=False,
                stop=True,
            )
            osb = osb_p.tile([128, D], f32, tag="osb", name=f"osbt{qt}")
            nc.vector.scalar_tensor_tensor(
                osb[:], ps[:], 1.0, bob,
                op0=mybir.AluOpType.mult, op1=mybir.AluOpType.add,
            )
            nc.sync.dma_start(d_out[qq : qq + 128, :], osb[:])

    nc.compile()
    return nc


def _prep_inputs(query, key, value, mask, wq_w, wq_b, wk_w, wk_b, wv_w, wv_b, wo_w, wo_b,
                 mm_dtype="bf16"):
    import ml_dtypes

    od = {"bf16": ml_dtypes.bfloat16, "f32r": np.float32, "f32": np.float32}[mm_dtype]
    f = lambda a: np.ascontiguousarray(np.asarray(a, dtype=np.float32))
    g = lambda a: np.ascontiguousarray(np.asarray(a).astype(od))
    query, key, value = f(query), f(key), f(value)
    wq_w, wk_w, wv_w, wo_w = f(wq_w), f(wk_w), f(wv_w), f(wo_w)
    mask = np.asarray(mask)

    keeps = [np.flatnonzero(mask[b] == 0) for b in range(B)]
    cnts = [len(k) for k in keeps]
    assert min(cnts) > 0, "all-masked batch not supported"
    s_pad = max(128, ((max(cnts) + 127) // 128) * 128)
    nsk = s_pad // 128

    bq_pp = np.ascontiguousarray(f(wq_b).reshape(4, 128).T)
    bk_pp = np.ascontiguousarray(f(wk_b).reshape(4, 128).T)
    bvb = np.broadcast_to(f(wv_b).reshape(1, D), (128, D))
    bob = np.broadcast_to(f(wo_b).reshape(1, D), (128, D))

    def wchunks(w):
        # [512, 512] -> [128, 4*512]: col block dk holds rows dk*128..dk*128+128
        return w.reshape(4, 128, D).transpose(1, 0, 2).reshape(128, 4 * D)

    def xchunks(xt):
        # [512, S] -> [128, 4*S]
        s = xt.shape[1]
        return xt.reshape(4, 128, s).transpose(1, 0, 2).reshape(128, 4 * s)

    blob_o = np.concatenate([wchunks(f(wo_w)), bob], axis=1)

    common = dict(
        bq_pp=bq_pp, bk_pp=bk_pp,
        blob_o=g(blob_o),
    )
    in_maps = []
    for b in range(B):
        kc = np.zeros((s_pad, D), np.float32)
        kc[: cnts[b]] = key[b][keeps[b]]
        vc = np.zeros((s_pad, D), np.float32)
        vc[: cnts[b]] = value[b][keeps[b]]
        blob_k = g(np.concatenate([wchunks(f(wk_w)), xchunks(kc.T)], axis=1))
        blob_v = g(
            np.concatenate(
                [
                    wchunks(f(wv_w)),
                    xchunks(vc.T),
                    bvb,
                    np.ones((128, 8), np.float32),
                ],
                axis=1,
            )
        )
        mbf = np.zeros(s_pad, np.float32)
        mbf[cnts[b] :] = -1e9
        mbd = np.ascontiguousarray(mbf.reshape(nsk, 128).T)
        for qh in range(2):
            blob_q = g(
                np.concatenate(
                    [wchunks(f(wq_w)), xchunks(query[b, qh * SQ : (qh + 1) * SQ, :].T)],
                    axis=1,
                )
            )
            in_maps.append(
                dict(blob_k=blob_k, blob_q=blob_q, blob_v=blob_v, mb=mbd, **common)
            )
    return s_pad, in_maps


def kernel(**inputs):
    from concourse import bass_utils

    mmd = os.environ.get("BASSK_MMDT", "bf16")
    s_pad, in_maps = _prep_inputs(**inputs, mm_dtype=mmd)
    key = (s_pad, mmd)
    if key not in _BUILT:
        _BUILT[key] = build_bass(s_pad, mm_dtype=key[1])
    nc = _BUILT[key]
    kw = {}
    if os.environ.get("BASSK_TRACE"):
        kw = dict(trace=True, stitch_traces=False)
    res = bass_utils.run_bass_kernel_spmd(nc, in_maps, core_ids=list(range(NCORE)), **kw)
    out = np.empty((B, S, D), np.float32)
    for c in range(NCORE):
        b, qh = c // 2, c % 2
        out[b, qh * SQ : (qh + 1) * SQ, :] = res.results[c]["out"]
    kernel.last_result = res
    return out
